# revision 27
# baseline (speedup 1.0000x reference)
"""Causal self-attention (B=2, T=2048, C=768, H=12) on 8 Trainium2 cores.

Sharding: 24 (batch, head) pairs / 8 cores = 3 heads per core.
core c -> batch b = c // 4, heads [3g, 3g+3) with g = c % 4.

Per-core device program (identical SPMD program, different input data):
  qkT  = (Wqk_local^T @ x_b^T)          [384, T]   (q cols pre-scaled 1/8,
                                                    q bias added, k bias
                                                    dropped: softmax-invariant)
  V    = x_b @ Wv_local                  [T, 192]   (v bias folded on host)
  per head h:
    scoresT[k, q] = kT_h^T-block @ qT_h  (PE, K=64; diagonal blocks trimmed
                                          to the causally-needed q columns)
    expT = exp(scoresT)                  (ACT; diagonal blocks multiplied
                                          by precomputed 0/1 masks on DVE,
                                          trimmed cols zero-filled)
    y_augT[[d;1], q] += V_aug^T @ expT   (PE, ones row -> softmax denom)
    yT_h = y_augT[y rows] * (1/denom)    (DVE approx-reciprocal; denom
                                          broadcast via gpsimd
                                          partition_broadcast)
  out_partial = Y_local @ Wp_local       [T, 768]   (emitted one q-chunk
                                          late to avoid PE head-of-line
                                          blocking on the yT writes)

Host: out[b] = sum of the 4 partials + (b_proj + b_v @ W_proj).

Matmuls run in float32r (single-pass fp32, ~13 mantissa bits, ~2.2x
faster than the two-pass LOW_HIGH fp32 mode). Set MM_DT to
mybir.dt.float32 to go back to exact fp32.

qkT feature-chunk layout (matmul needs lhsT/rhs on the same base
partition, so each head's q and k live at the same partition offset):
  chunk0 = [q0 | q2], chunk1 = [k0 | k2], chunk2 = [q1], chunk3 = [k1]
yT layout [128, 2, T]: h0 -> (0:64, 0), h1 -> (64:128, 0), h2 -> (0:64, 1)
so the out-projection fuses h0+h1 into one K=128 matmul.
V_aug per-kb free layout [65 | 128 | 65]:
  h0: [v_h0, 1]; h1: [1, 0*63, v_h1] (y rows 64:128, denom row 0);
  h2: [v_h2, 1]
"""

import numpy as np

import concourse.bass as bass
import concourse.mybir as mybir
import concourse.tile as tile
from concourse import bacc
from concourse import bass_utils

P = 128
D = 64          # head dim
HPC = 3         # heads per core
C = 768
CK = C // P     # 6 contraction chunks
QK = 2 * HPC * D  # 384 (q+k cols per core)
NH = 12
B = 2
N_CORES = 8
F32 = mybir.dt.float32
MM_DT = mybir.dt.float32r

# (partition offset, chunk idx) per head, for q and k
Q_POS = [(0, 0), (0, 2), (64, 0)]
K_POS = [(0, 1), (0, 3), (64, 1)]
# wqk DRAM column ranges per chunk: (start, width)
QK_CHUNKS = [(0, 128), (128, 128), (256, 64), (320, 64)]
# V_aug free-layout per head: (lhsT start, lhsT width, denom row, y row0)
V_SLICE = [(0, 65, 64, 0), (65, 128, 0, 64), (193, 65, 64, 0)]
VW = 258
# yT destination (row0, chunk) per head
Y_POS = [(0, 0), (64, 0), (0, 1)]


def build_nc(T=2048, QCW=512):
    """Build the per-core Bass program. T = sequence length, QCW = q-chunk."""
    assert T % QCW == 0 and QCW % P == 0 and T % 512 == 0
    NQC = T // QCW
    NTB = T // P
    NPH = C // 2  # 384, out-proj free-dim half

    nc = bacc.Bacc("TRN2", target_bir_lowering=False, debug=False,
                   num_devices=N_CORES)
    xT = nc.dram_tensor("xT", [C, T], F32, kind="ExternalInput").ap()
    wqk = nc.dram_tensor("wqk", [C, QK], F32, kind="ExternalInput").ap()
    wv = nc.dram_tensor("wv", [C, HPC * D], F32, kind="ExternalInput").ap()
    bqk = nc.dram_tensor("bqk", [512], F32, kind="ExternalInput").ap()
    wp = nc.dram_tensor("wp", [2 * P, C], F32, kind="ExternalInput").ap()
    out = nc.dram_tensor("out", [T, C], F32, kind="ExternalOutput").ap()

    Exp = mybir.ActivationFunctionType.Exp

    with tile.TileContext(nc) as tc:
        with (
            tc.tile_pool(name="const", bufs=1) as const,
            tc.tile_pool(name="work", bufs=4) as work,
            tc.tile_pool(name="small", bufs=2) as small,
            tc.tile_pool(name="outp", bufs=3) as outp,
            tc.tile_pool(name="ps_mm", bufs=5, space="PSUM") as ps_mm,
            tc.tile_pool(name="ps_y", bufs=3, space="PSUM") as ps_y_pool,
        ):
            xT_sb = const.tile([P, CK, T], MM_DT, tag="xT")
            wqk_sb = const.tile([P, CK, QK], MM_DT, tag="wqk")
            wv_sb = const.tile([P, CK, HPC * D], MM_DT, tag="wv")
            bqk_sb = const.tile([P, 4], F32, tag="bqk")
            wp_sb = const.tile([P, 2, C], MM_DT, tag="wp")
            qkT_sb = const.tile([P, 4, T], MM_DT, tag="qkT")
            v_sb = const.tile([P, NTB, VW], MM_DT, tag="v")
            yT_sb = const.tile([P, 2, T], MM_DT, tag="yT")
            zb_sb = const.tile([P, 1], F32, tag="zb")
            ones_sb = const.tile([1, P], MM_DT, tag="ones")
            mask_sb = const.tile([P, 4, QCW], F32, tag="mask")
            zq_sb = const.tile([P, 512], F32, tag="zq")

            # ---- loads (weights first; xT per (kc, tj) chunk, spread
            # across the sync/gpsimd/scalar DMA queues) ----
            dma_engs = [nc.sync, nc.gpsimd, nc.scalar]
            # per-kc (wqk, xT-slice0) pairs interleaved at the head of all
            # three queues so the first qkT matmul chain starts ~2us in,
            # instead of waiting behind a monolithic 1.15MB wqk transfer
            nc.scalar.dma_start(bqk_sb[:], bqk.rearrange("(ci p) -> p ci", p=P))
            for kc in range(CK):
                dma_engs[(2 * kc) % 3].dma_start(
                    wqk_sb[:, kc, :],
                    wqk[kc * P:(kc + 1) * P, :].bitcast(MM_DT))
                dma_engs[(2 * kc + 1) % 3].dma_start(
                    xT_sb[:, kc, 0:512],
                    xT[kc * P:(kc + 1) * P, 0:512].bitcast(MM_DT))
            nc.gpsimd.dma_start(wv_sb[:], wv.bitcast(MM_DT).rearrange("(kc p) m -> p kc m", p=P))
            di = 0
            for tj in range(1, T // 512):
                for kc in range(CK):
                    dma_engs[di % 3].dma_start(
                        xT_sb[:, kc, tj * 512:(tj + 1) * 512],
                        xT[kc * P:(kc + 1) * P,
                           tj * 512:(tj + 1) * 512].bitcast(MM_DT))
                    di += 1
            # wp is first needed by proj_qc(0), long after the xT stream
            nc.scalar.dma_start(wp_sb[:], wp.bitcast(MM_DT).rearrange("(ci p) e -> p ci e", p=P))

            # ---- constants (f32r tiles can't be memset; stage + cast) ----
            st = const.tile([P, 2], F32, tag="st")
            st1 = const.tile([1, P], F32, tag="st1")
            nc.gpsimd.memset(st[:], 1.0)
            nc.gpsimd.memset(st1[:], 1.0)
            nc.vector.tensor_copy(ones_sb[:], st1[:])
            nc.vector.tensor_copy(v_sb[:, :, 64:66],
                                  st[:, None, :].to_broadcast((P, NTB, 2)))
            nc.vector.tensor_copy(v_sb[:, :, 257:258],
                                  st[:, None, 0:1].to_broadcast((P, NTB, 1)))
            nc.gpsimd.memset(zq_sb[:], 0.0)
            # h1 junk cols (feed only never-read psy rows); zero for sim
            nc.vector.tensor_copy(v_sb[:, :, 66:129],
                                  zq_sb[:, None, 0:63].to_broadcast((P, NTB, 63)))
            nc.gpsimd.memset(zb_sb[:], 0.0)
            # causal 0/1 masks: mask_j[x, y] = 1 if y - x >= 128*j else 0
            nc.gpsimd.memset(mask_sb[:], 1.0)
            for j in range(4):
                nc.gpsimd.affine_select(
                    mask_sb[:, j, :], mask_sb[:, j, :],
                    pattern=[[1, QCW]],
                    compare_op=mybir.AluOpType.is_ge,
                    fill=0.0,
                    base=-128 * j,
                    channel_multiplier=-1,
                )

            # ---- qkT + V projections for one 512-token slice.
            # chunks 2+3 ([q1|k1]) fused into one 128-row matmul chain; the
            # k1 half is relocated to chunk 3 partitions 0:64 by an
            # SBUF->SBUF DMA (engines can't cross partitions, DMA can). ----
            def project_tj(tj):
                tjs = slice(tj * 512, (tj + 1) * 512)
                for ci in range(2):
                    ps = ps_mm.tile([P, 512], F32, tag="mm")
                    for kc in range(CK):
                        nc.tensor.matmul(
                            ps[:],
                            wqk_sb[:, kc, ci * P:(ci + 1) * P],
                            xT_sb[:, kc, tjs],
                            start=(kc == 0), stop=(kc == CK - 1),
                        )
                    nc.vector.tensor_scalar_add(
                        qkT_sb[:, ci, tjs], ps[:], bqk_sb[:, ci:ci + 1])
                ps = ps_mm.tile([P, 512], F32, tag="mm")
                for kc in range(CK):
                    nc.tensor.matmul(
                        ps[:],
                        wqk_sb[:, kc, 256:384],
                        xT_sb[:, kc, tjs],
                        start=(kc == 0), stop=(kc == CK - 1),
                    )
                nc.vector.tensor_scalar_add(
                    qkT_sb[0:D, 2, tjs], ps[0:D, :], bqk_sb[0:D, 2:3])
                qkst = work.tile([P, 512], MM_DT, tag="qkst")
                nc.vector.tensor_copy(qkst[D:P, :], ps[D:P, :])
                nc.sync.dma_start(qkT_sb[0:D, 3, tjs], qkst[D:P, :])
                for tb in range(4 * tj, 4 * tj + 4):
                    ps = ps_mm.tile([P, 512], F32, tag="mm")
                    for kc in range(CK):
                        nc.tensor.matmul(
                            ps[:, :HPC * D],
                            xT_sb[:, kc, tb * P:(tb + 1) * P],
                            wv_sb[:, kc, :],
                            start=(kc == 0), stop=(kc == CK - 1),
                        )
                    nc.vector.tensor_copy(v_sb[:, tb, 0:64], ps[:, 0:64])
                    nc.vector.tensor_copy(v_sb[:, tb, 129:257],
                                          ps[:, 64:192])

            # ---- attention (per q-chunk) and delayed out-projection.
            # Normalization of head i is emitted during head i+1's matmul
            # loop so its ACT->DVE->PE chain never stalls the in-order PE.
            pend1, pend2 = [], []

            def norm_stage1(st8):
                qc, h, psy_t = st8
                v0, vw, srow, yrow = V_SLICE[h]
                den = small.tile([1, QCW], F32, tag="den")
                nc.scalar.copy(den[:], psy_t[srow:srow + 1, :])
                recf = small.tile([1, QCW], F32, tag="recf")
                nc.vector.reciprocal_approx_fast(recf[:], den[:])
                recip = small.tile([1, QCW], MM_DT, tag="recip")
                nc.vector.tensor_copy(recip[:], recf[:])
                return (qc, h, psy_t, recip)

            def norm_stage2(st8):
                qc, h, psy_t, recip = st8
                q0 = qc * QCW
                v0, vw, srow, yrow = V_SLICE[h]
                psb = ps_mm.tile([P, QCW], F32, tag="mm", name="psb")
                nc.tensor.matmul(psb[:], ones_sb[:], recip[:],
                                 start=True, stop=True)
                bc = small.tile([P, QCW], F32, tag="bcs")
                yp, yci = Y_POS[h]
                nc.scalar.copy(bc[yrow:yrow + D, :],
                               psb[yrow:yrow + D, :])
                nc.vector.tensor_mul(
                    yT_sb[yp:yp + D, yci, q0:q0 + QCW],
                    psy_t[yrow:yrow + D, :], bc[yrow:yrow + D, :])

            def attn_qc(qc):
                q0 = qc * QCW
                kbmax = (q0 + QCW - 1) // P
                for h in range(HPC):
                    qp, qci = Q_POS[h]
                    kp, kci = K_POS[h]
                    v0, vw, srow, yrow = V_SLICE[h]
                    psy_t = ps_y_pool.tile([P, QCW], F32, tag="yaug",
                                           name="psy")
                    psy = psy_t[0:vw, :]
                    for kb in range(kbmax + 1):
                        # diagonal blocks only need q columns >= kb*128
                        n0 = max(0, kb * P - q0)
                        pss = ps_mm.tile([P, QCW], F32, tag="mm")
                        nc.tensor.matmul(
                            pss[:, n0:],
                            qkT_sb[kp:kp + D, kci, kb * P:(kb + 1) * P],
                            qkT_sb[qp:qp + D, qci, q0 + n0:q0 + QCW],
                            start=True, stop=True,
                        )
                        expT = work.tile([P, QCW], MM_DT, tag="expT")
                        nc.scalar.activation(expT[:, n0:], pss[:, n0:], Exp,
                                             bias=zb_sb[:])
                        if kb * P >= q0:  # diagonal block
                            nc.vector.tensor_mul(
                                expT[:, n0:], expT[:, n0:],
                                mask_sb[:, kb - q0 // P, n0:])
                        nc.tensor.matmul(
                            psy[:, n0:], v_sb[:, kb, v0:v0 + vw],
                            expT[:, n0:],
                            start=(kb == 0), stop=(kb == kbmax),
                        )
                        if kb == 0 and pend1:
                            pend2.append(norm_stage1(pend1.pop(0)))
                        if kb == 3 and pend2:
                            norm_stage2(pend2.pop(0))
                    pend1.append((qc, h, psy_t))

            def proj_qc(qc):
                q0 = qc * QCW
                for tb in range(q0 // P, (q0 + QCW) // P):
                    osb = outp.tile([P, C], F32, tag="osb")
                    for half in range(2):
                        pso = ps_mm.tile([P, 512], F32, tag="mm",
                                         name="pso")[:, :NPH]
                        nc.tensor.matmul(
                            pso, yT_sb[:, 0, tb * P:(tb + 1) * P],
                            wp_sb[:, 0, half * NPH:(half + 1) * NPH],
                            start=True, stop=False)
                        nc.tensor.matmul(
                            pso, yT_sb[0:D, 1, tb * P:(tb + 1) * P],
                            wp_sb[0:D, 1, half * NPH:(half + 1) * NPH],
                            start=False, stop=True)
                        nc.vector.tensor_copy(
                            osb[:, half * NPH:(half + 1) * NPH], pso)
                    dma_engs[tb % 3].dma_start(
                        out[tb * P:(tb + 1) * P, :], osb[:])

            # interleaved pipeline: projections(tj) -> attention(tj) ->
            # out-projection(tj-1), so PE never queues behind a later
            # slice's xT DMA
            for tj in range(T // 512):
                project_tj(tj)
                attn_qc(tj)
                if tj > 0:
                    proj_qc(tj - 1)
            while pend1:
                pend2.append(norm_stage1(pend1.pop(0)))
            while pend2:
                norm_stage2(pend2.pop(0))
            proj_qc(NQC - 1)


    nc.compile()
    return nc


_NC_CACHE = {}


def _get_nc(T=2048, QCW=512):
    key = (T, QCW)
    if key not in _NC_CACHE:
        _NC_CACHE[key] = build_nc(T, QCW)
    return _NC_CACHE[key]


def build_in_maps(inputs):
    """Build the 8 per-core input dicts from full inputs."""
    x = np.asarray(inputs["x"], np.float32)
    W = np.asarray(inputs["W_attn"], np.float32)
    b = np.asarray(inputs["b_attn"], np.float32)
    W_proj = np.asarray(inputs["W_proj"], np.float32)
    in_maps = []
    for c in range(N_CORES):
        bi, g = divmod(c, 4)
        lo = g * (HPC * D)  # local head col offset within each of q/k/v
        qw = [W[:, lo + i * D:lo + (i + 1) * D] * 0.125 for i in range(HPC)]
        kw = [W[:, C + lo + i * D:C + lo + (i + 1) * D] for i in range(HPC)]
        qb = [b[lo + i * D:lo + (i + 1) * D] * 0.125 for i in range(HPC)]
        # chunk order: [q0|q2], [k0|k2], [q1], [k1]
        wqk = np.concatenate([qw[0], qw[2], kw[0], kw[2], qw[1], kw[1]],
                             axis=1)
        z64 = np.zeros(D, np.float32)
        bqk = np.concatenate([qb[0], qb[2], z64, z64, qb[1], z64, z64, z64])
        wv = W[:, 2 * C + lo:2 * C + lo + HPC * D]
        # wp rows: [h0 | h1 | h2 | zero pad] -> chunks (0:128), (128:256)
        wp = np.zeros((2 * P, C), np.float32)
        wp[:HPC * D] = W_proj[lo:lo + HPC * D]
        in_maps.append({
            "xT": np.ascontiguousarray(x[bi].T),
            "wqk": np.ascontiguousarray(wqk),
            "wv": np.ascontiguousarray(wv),
            "bqk": np.ascontiguousarray(bqk),
            "wp": np.ascontiguousarray(wp),
        })
    return in_maps


def postprocess(results, inputs):
    b_attn = np.asarray(inputs["b_attn"], np.float32)
    W_proj = np.asarray(inputs["W_proj"], np.float32)
    b_proj = np.asarray(inputs["b_proj"], np.float32)
    b_eff = b_proj + b_attn[2 * C:] @ W_proj
    T = results[0]["out"].shape[0]
    out = np.zeros((B, T, C), np.float32)
    for c in range(N_CORES):
        out[c // 4] += results[c]["out"]
    out += b_eff
    return out


def kernel(x, W_attn, b_attn, W_proj, b_proj):
    inputs = dict(x=x, W_attn=W_attn, b_attn=b_attn,
                  W_proj=W_proj, b_proj=b_proj)
    T = np.asarray(x).shape[1]
    nc = _get_nc(T=T)
    in_maps = build_in_maps(inputs)
    res = bass_utils.run_bass_kernel_spmd(
        nc, in_maps, core_ids=list(range(N_CORES)))
    return postprocess(res.results, inputs)


# revision 29
# speedup vs baseline: 1.0382x; 1.0382x over previous
"""Causal self-attention (B=2, T=2048, C=768, H=12) on 8 Trainium2 cores.

Sharding: 24 (batch, head) pairs / 8 cores = 3 heads per core.
core c -> batch b = c // 4, heads [3g, 3g+3) with g = c % 4.

Per-core device program (identical SPMD program, different input data):
  qkT  = (Wqk_local^T @ x_b^T)          [384, T]   (q cols pre-scaled 1/8,
                                                    q bias added, k bias
                                                    dropped: softmax-invariant)
  V    = x_b @ Wv_local                  [T, 192]   (v bias folded on host)
  per head h:
    scoresT[k, q] = kT_h^T-block @ qT_h  (PE, K=64; diagonal blocks trimmed
                                          to the causally-needed q columns)
    expT = exp(scoresT)                  (ACT; diagonal blocks multiplied
                                          by precomputed 0/1 masks on DVE,
                                          trimmed cols zero-filled)
    y_augT[[d;1], q] += V_aug^T @ expT   (PE, ones row -> softmax denom)
    yT_h = y_augT[y rows] * (1/denom)    (DVE approx-reciprocal; denom
                                          broadcast via gpsimd
                                          partition_broadcast)
  out_partial = Y_local @ Wp_local       [T, 768]   (emitted one q-chunk
                                          late to avoid PE head-of-line
                                          blocking on the yT writes)

Host: out[b] = sum of the 4 partials + (b_proj + b_v @ W_proj).

Matmuls run in float32r (single-pass fp32, ~13 mantissa bits, ~2.2x
faster than the two-pass LOW_HIGH fp32 mode). Set MM_DT to
mybir.dt.float32 to go back to exact fp32.

qkT feature-chunk layout (matmul needs lhsT/rhs on the same base
partition, so each head's q and k live at the same partition offset):
  chunk0 = [q0 | q2], chunk1 = [k0 | k2], chunk2 = [q1], chunk3 = [k1]
yT layout [128, 2, T]: h0 -> (0:64, 0), h1 -> (64:128, 0), h2 -> (0:64, 1)
so the out-projection fuses h0+h1 into one K=128 matmul.
V_aug per-kb free layout [65 | 128 | 65]:
  h0: [v_h0, 1]; h1: [1, 0*63, v_h1] (y rows 64:128, denom row 0);
  h2: [v_h2, 1]
"""

import numpy as np

import concourse.bass as bass
import concourse.mybir as mybir
import concourse.tile as tile
from concourse import bacc
from concourse import bass_utils

P = 128
D = 64          # head dim
HPC = 3         # heads per core
C = 768
CK = C // P     # 6 contraction chunks
QK = 2 * HPC * D  # 384 (q+k cols per core)
NH = 12
B = 2
N_CORES = 8
F32 = mybir.dt.float32
MM_DT = mybir.dt.float32r

# (partition offset, chunk idx) per head, for q and k
Q_POS = [(0, 0), (0, 2), (64, 0)]
K_POS = [(0, 1), (0, 3), (64, 1)]
# wqk DRAM column ranges per chunk: (start, width)
QK_CHUNKS = [(0, 128), (128, 128), (256, 64), (320, 64)]
# V_aug free-layout per head: (lhsT start, lhsT width, denom row, y row0)
V_SLICE = [(0, 65, 64, 0), (65, 128, 0, 64), (193, 65, 64, 0)]
VW = 258
# yT destination (row0, chunk) per head
Y_POS = [(0, 0), (64, 0), (0, 1)]


def build_nc(T=2048, QCW=512):
    """Build the per-core Bass program. T = sequence length, QCW = q-chunk."""
    assert T % QCW == 0 and QCW % P == 0 and T % 512 == 0
    NQC = T // QCW
    NTB = T // P
    NPH = C // 2  # 384, out-proj free-dim half

    nc = bacc.Bacc("TRN2", target_bir_lowering=False, debug=False,
                   num_devices=N_CORES)
    xT = nc.dram_tensor("xT", [C, T], F32, kind="ExternalInput").ap()
    wqk = nc.dram_tensor("wqk", [C, QK], F32, kind="ExternalInput").ap()
    wv = nc.dram_tensor("wv", [C, HPC * D], F32, kind="ExternalInput").ap()
    bqk = nc.dram_tensor("bqk", [512], F32, kind="ExternalInput").ap()
    wp = nc.dram_tensor("wp", [2 * P, C], F32, kind="ExternalInput").ap()
    out = nc.dram_tensor("out", [T, C], F32, kind="ExternalOutput").ap()

    Exp = mybir.ActivationFunctionType.Exp

    with tile.TileContext(nc) as tc:
        with (
            tc.tile_pool(name="const", bufs=1) as const,
            tc.tile_pool(name="work", bufs=4) as work,
            tc.tile_pool(name="small", bufs=2) as small,
            tc.tile_pool(name="outp", bufs=3) as outp,
            tc.tile_pool(name="ps_mm", bufs=6, space="PSUM") as ps_mm,
            tc.tile_pool(name="ps_y", bufs=2, space="PSUM") as ps_y_pool,
        ):
            xT_sb = const.tile([P, CK, T], MM_DT, tag="xT")
            wqk_sb = const.tile([P, CK, QK], MM_DT, tag="wqk")
            wv_sb = const.tile([P, CK, HPC * D], MM_DT, tag="wv")
            bqk_sb = const.tile([P, 4], F32, tag="bqk")
            wp_sb = const.tile([P, 2, C], MM_DT, tag="wp")
            qkT_sb = const.tile([P, 4, T], MM_DT, tag="qkT")
            v_sb = const.tile([P, NTB, VW], MM_DT, tag="v")
            yT_sb = const.tile([P, 2, T], MM_DT, tag="yT")
            zb_sb = const.tile([P, 1], F32, tag="zb")
            ones_sb = const.tile([1, P], MM_DT, tag="ones")
            mask_sb = const.tile([P, 4, QCW], F32, tag="mask")
            zq_sb = const.tile([P, 512], F32, tag="zq")

            # ---- loads (weights first; xT per (kc, tj) chunk, spread
            # across the sync/gpsimd/scalar DMA queues) ----
            dma_engs = [nc.sync, nc.gpsimd, nc.scalar]
            # per-kc (wqk, xT-slice0) pairs interleaved at the head of all
            # three queues so the first qkT matmul chain starts ~2us in,
            # instead of waiting behind a monolithic 1.15MB wqk transfer
            nc.scalar.dma_start(bqk_sb[:], bqk.rearrange("(ci p) -> p ci", p=P))
            for kc in range(CK):
                dma_engs[(2 * kc) % 3].dma_start(
                    wqk_sb[:, kc, :],
                    wqk[kc * P:(kc + 1) * P, :].bitcast(MM_DT))
                dma_engs[(2 * kc + 1) % 3].dma_start(
                    xT_sb[:, kc, 0:512],
                    xT[kc * P:(kc + 1) * P, 0:512].bitcast(MM_DT))
            nc.gpsimd.dma_start(wv_sb[:], wv.bitcast(MM_DT).rearrange("(kc p) m -> p kc m", p=P))
            nc.scalar.dma_start(wp_sb[:], wp.bitcast(MM_DT).rearrange("(ci p) e -> p ci e", p=P))
            di = 0
            for tj in range(1, T // 512):
                for kc in range(CK):
                    dma_engs[di % 3].dma_start(
                        xT_sb[:, kc, tj * 512:(tj + 1) * 512],
                        xT[kc * P:(kc + 1) * P,
                           tj * 512:(tj + 1) * 512].bitcast(MM_DT))
                    di += 1

            # ---- constants (f32r tiles can't be memset; stage + cast) ----
            st = const.tile([P, 2], F32, tag="st")
            st1 = const.tile([1, P], F32, tag="st1")
            nc.gpsimd.memset(st[:], 1.0)
            nc.gpsimd.memset(st1[:], 1.0)
            nc.vector.tensor_copy(ones_sb[:], st1[:])
            nc.vector.tensor_copy(v_sb[:, :, 64:66],
                                  st[:, None, :].to_broadcast((P, NTB, 2)))
            nc.vector.tensor_copy(v_sb[:, :, 257:258],
                                  st[:, None, 0:1].to_broadcast((P, NTB, 1)))
            nc.gpsimd.memset(zq_sb[:], 0.0)
            # h1 junk cols (feed only never-read psy rows); zero for sim
            nc.vector.tensor_copy(v_sb[:, :, 66:129],
                                  zq_sb[:, None, 0:63].to_broadcast((P, NTB, 63)))
            nc.gpsimd.memset(zb_sb[:], 0.0)
            # causal 0/1 masks: mask_j[x, y] = 1 if y - x >= 128*j else 0
            nc.gpsimd.memset(mask_sb[:], 1.0)
            for j in range(4):
                nc.gpsimd.affine_select(
                    mask_sb[:, j, :], mask_sb[:, j, :],
                    pattern=[[1, QCW]],
                    compare_op=mybir.AluOpType.is_ge,
                    fill=0.0,
                    base=-128 * j,
                    channel_multiplier=-1,
                )

            # ---- qkT + V projections for one 512-token slice.
            # chunks 2+3 ([q1|k1]) fused into one 128-row matmul chain; the
            # k1 half is relocated to chunk 3 partitions 0:64 by an
            # SBUF->SBUF DMA (engines can't cross partitions, DMA can). ----
            def project_tj(tj):
                tjs = slice(tj * 512, (tj + 1) * 512)
                for ci in range(2):
                    ps = ps_mm.tile([P, 512], F32, tag="mm")
                    for kc in range(CK):
                        nc.tensor.matmul(
                            ps[:],
                            wqk_sb[:, kc, ci * P:(ci + 1) * P],
                            xT_sb[:, kc, tjs],
                            start=(kc == 0), stop=(kc == CK - 1),
                        )
                    nc.vector.tensor_scalar_add(
                        qkT_sb[:, ci, tjs], ps[:], bqk_sb[:, ci:ci + 1])
                ps = ps_mm.tile([P, 512], F32, tag="mm")
                for kc in range(CK):
                    nc.tensor.matmul(
                        ps[:],
                        wqk_sb[:, kc, 256:384],
                        xT_sb[:, kc, tjs],
                        start=(kc == 0), stop=(kc == CK - 1),
                    )
                nc.vector.tensor_scalar_add(
                    qkT_sb[0:D, 2, tjs], ps[0:D, :], bqk_sb[0:D, 2:3])
                qkst = work.tile([P, 512], MM_DT, tag="qkst")
                nc.vector.tensor_copy(qkst[D:P, :], ps[D:P, :])
                nc.sync.dma_start(qkT_sb[0:D, 3, tjs], qkst[D:P, :])
                for tb in range(4 * tj, 4 * tj + 4):
                    ps = ps_mm.tile([P, 512], F32, tag="mm")
                    for kc in range(CK):
                        nc.tensor.matmul(
                            ps[:, :HPC * D],
                            xT_sb[:, kc, tb * P:(tb + 1) * P],
                            wv_sb[:, kc, :],
                            start=(kc == 0), stop=(kc == CK - 1),
                        )
                    nc.vector.tensor_copy(v_sb[:, tb, 0:64], ps[:, 0:64])
                    nc.vector.tensor_copy(v_sb[:, tb, 129:257],
                                          ps[:, 64:192])

            # ---- attention (per q-chunk) and delayed out-projection.
            # Normalization of head i is emitted during head i+1's matmul
            # loop so its ACT->DVE->PE chain never stalls the in-order PE.
            pend1, pend2 = [], []

            def norm_stage1(st8):
                qc, h, psy_t = st8
                v0, vw, srow, yrow = V_SLICE[h]
                den = small.tile([1, QCW], F32, tag="den")
                nc.scalar.copy(den[:], psy_t[srow:srow + 1, :])
                recf = small.tile([1, QCW], F32, tag="recf")
                nc.vector.reciprocal_approx_fast(recf[:], den[:])
                recip = small.tile([1, QCW], MM_DT, tag="recip")
                nc.vector.tensor_copy(recip[:], recf[:])
                return (qc, h, psy_t, recip)

            def norm_stage2(st8):
                qc, h, psy_t, recip = st8
                q0 = qc * QCW
                v0, vw, srow, yrow = V_SLICE[h]
                psb = ps_mm.tile([P, QCW], F32, tag="mm", name="psb")
                nc.tensor.matmul(psb[:], ones_sb[:], recip[:],
                                 start=True, stop=True)
                bc = small.tile([P, QCW], F32, tag="bcs")
                yp, yci = Y_POS[h]
                nc.scalar.copy(bc[yrow:yrow + D, :],
                               psb[yrow:yrow + D, :])
                nc.vector.tensor_mul(
                    yT_sb[yp:yp + D, yci, q0:q0 + QCW],
                    psy_t[yrow:yrow + D, :], bc[yrow:yrow + D, :])

            def attn_qc(qc):
                q0 = qc * QCW
                kbmax = (q0 + QCW - 1) // P
                for h in range(HPC):
                    qp, qci = Q_POS[h]
                    kp, kci = K_POS[h]
                    v0, vw, srow, yrow = V_SLICE[h]
                    psy_t = ps_y_pool.tile([P, QCW], F32, tag="yaug",
                                           name="psy")
                    psy = psy_t[0:vw, :]
                    for kb in range(kbmax + 1):
                        # diagonal blocks only need q columns >= kb*128
                        n0 = max(0, kb * P - q0)
                        pss = ps_mm.tile([P, QCW], F32, tag="mm")
                        nc.tensor.matmul(
                            pss[:, n0:],
                            qkT_sb[kp:kp + D, kci, kb * P:(kb + 1) * P],
                            qkT_sb[qp:qp + D, qci, q0 + n0:q0 + QCW],
                            start=True, stop=True,
                        )
                        expT = work.tile([P, QCW], MM_DT, tag="expT")
                        nc.scalar.activation(expT[:, n0:], pss[:, n0:], Exp,
                                             bias=zb_sb[:])
                        if kb * P >= q0:  # diagonal block
                            nc.vector.tensor_mul(
                                expT[:, n0:], expT[:, n0:],
                                mask_sb[:, kb - q0 // P, n0:])
                        nc.tensor.matmul(
                            psy[:, n0:], v_sb[:, kb, v0:v0 + vw],
                            expT[:, n0:],
                            start=(kb == 0), stop=(kb == kbmax),
                        )
                        if kb == 0 and pend1:
                            pend2.append(norm_stage1(pend1.pop(0)))
                        if kb == 3 and pend2:
                            norm_stage2(pend2.pop(0))
                    pend1.append((qc, h, psy_t))

            def proj_qc(qc):
                q0 = qc * QCW
                for tb in range(q0 // P, (q0 + QCW) // P):
                    osb = outp.tile([P, C], F32, tag="osb")
                    for half in range(2):
                        pso = ps_mm.tile([P, 512], F32, tag="mm",
                                         name="pso")[:, :NPH]
                        nc.tensor.matmul(
                            pso, yT_sb[:, 0, tb * P:(tb + 1) * P],
                            wp_sb[:, 0, half * NPH:(half + 1) * NPH],
                            start=True, stop=False)
                        nc.tensor.matmul(
                            pso, yT_sb[0:D, 1, tb * P:(tb + 1) * P],
                            wp_sb[0:D, 1, half * NPH:(half + 1) * NPH],
                            start=False, stop=True)
                        nc.vector.tensor_copy(
                            osb[:, half * NPH:(half + 1) * NPH], pso)
                    dma_engs[tb % 3].dma_start(
                        out[tb * P:(tb + 1) * P, :], osb[:])

            # interleaved pipeline: projections(tj) -> attention(tj) ->
            # out-projection(tj-1), so PE never queues behind a later
            # slice's xT DMA
            for tj in range(T // 512):
                project_tj(tj)
                attn_qc(tj)
                if tj > 0:
                    proj_qc(tj - 1)
            while pend1:
                pend2.append(norm_stage1(pend1.pop(0)))
            while pend2:
                norm_stage2(pend2.pop(0))
            proj_qc(NQC - 1)


    nc.compile()
    return nc


_NC_CACHE = {}


def _get_nc(T=2048, QCW=512):
    key = (T, QCW)
    if key not in _NC_CACHE:
        _NC_CACHE[key] = build_nc(T, QCW)
    return _NC_CACHE[key]


def build_in_maps(inputs):
    """Build the 8 per-core input dicts from full inputs."""
    x = np.asarray(inputs["x"], np.float32)
    W = np.asarray(inputs["W_attn"], np.float32)
    b = np.asarray(inputs["b_attn"], np.float32)
    W_proj = np.asarray(inputs["W_proj"], np.float32)
    in_maps = []
    for c in range(N_CORES):
        bi, g = divmod(c, 4)
        lo = g * (HPC * D)  # local head col offset within each of q/k/v
        qw = [W[:, lo + i * D:lo + (i + 1) * D] * 0.125 for i in range(HPC)]
        kw = [W[:, C + lo + i * D:C + lo + (i + 1) * D] for i in range(HPC)]
        qb = [b[lo + i * D:lo + (i + 1) * D] * 0.125 for i in range(HPC)]
        # chunk order: [q0|q2], [k0|k2], [q1], [k1]
        wqk = np.concatenate([qw[0], qw[2], kw[0], kw[2], qw[1], kw[1]],
                             axis=1)
        z64 = np.zeros(D, np.float32)
        bqk = np.concatenate([qb[0], qb[2], z64, z64, qb[1], z64, z64, z64])
        wv = W[:, 2 * C + lo:2 * C + lo + HPC * D]
        # wp rows: [h0 | h1 | h2 | zero pad] -> chunks (0:128), (128:256)
        wp = np.zeros((2 * P, C), np.float32)
        wp[:HPC * D] = W_proj[lo:lo + HPC * D]
        in_maps.append({
            "xT": np.ascontiguousarray(x[bi].T),
            "wqk": np.ascontiguousarray(wqk),
            "wv": np.ascontiguousarray(wv),
            "bqk": np.ascontiguousarray(bqk),
            "wp": np.ascontiguousarray(wp),
        })
    return in_maps


def postprocess(results, inputs):
    b_attn = np.asarray(inputs["b_attn"], np.float32)
    W_proj = np.asarray(inputs["W_proj"], np.float32)
    b_proj = np.asarray(inputs["b_proj"], np.float32)
    b_eff = b_proj + b_attn[2 * C:] @ W_proj
    T = results[0]["out"].shape[0]
    out = np.zeros((B, T, C), np.float32)
    for c in range(N_CORES):
        out[c // 4] += results[c]["out"]
    out += b_eff
    return out


def kernel(x, W_attn, b_attn, W_proj, b_proj):
    inputs = dict(x=x, W_attn=W_attn, b_attn=b_attn,
                  W_proj=W_proj, b_proj=b_proj)
    T = np.asarray(x).shape[1]
    nc = _get_nc(T=T)
    in_maps = build_in_maps(inputs)
    res = bass_utils.run_bass_kernel_spmd(
        nc, in_maps, core_ids=list(range(N_CORES)))
    return postprocess(res.results, inputs)


# revision 30
# speedup vs baseline: 1.0600x; 1.0211x over previous
"""Causal self-attention (B=2, T=2048, C=768, H=12) on 8 Trainium2 cores.

Sharding: 24 (batch, head) pairs / 8 cores = 3 heads per core.
core c -> batch b = c // 4, heads [3g, 3g+3) with g = c % 4.

Per-core device program (identical SPMD program, different input data):
  qkT  = (Wqk_local^T @ x_b^T)          [384, T]   (q cols pre-scaled 1/8,
                                                    q bias added, k bias
                                                    dropped: softmax-invariant)
  V    = x_b @ Wv_local                  [T, 192]   (v bias folded on host)
  per head h:
    scoresT[k, q] = kT_h^T-block @ qT_h  (PE, K=64; diagonal blocks trimmed
                                          to the causally-needed q columns)
    expT = exp(scoresT)                  (ACT; diagonal blocks multiplied
                                          by precomputed 0/1 masks on DVE,
                                          trimmed cols zero-filled)
    y_augT[[d;1], q] += V_aug^T @ expT   (PE, ones row -> softmax denom)
    yT_h = y_augT[y rows] * (1/denom)    (DVE approx-reciprocal; denom
                                          broadcast via gpsimd
                                          partition_broadcast)
  out_partial = Y_local @ Wp_local       [T, 768]   (emitted one q-chunk
                                          late to avoid PE head-of-line
                                          blocking on the yT writes)

Host: out[b] = sum of the 4 partials + (b_proj + b_v @ W_proj).

Matmuls run in float32r (single-pass fp32, ~13 mantissa bits, ~2.2x
faster than the two-pass LOW_HIGH fp32 mode). Set MM_DT to
mybir.dt.float32 to go back to exact fp32.

qkT feature-chunk layout (matmul needs lhsT/rhs on the same base
partition, so each head's q and k live at the same partition offset):
  chunk0 = [q0 | q2], chunk1 = [k0 | k2], chunk2 = [q1], chunk3 = [k1]
yT layout [128, 2, T]: h0 -> (0:64, 0), h1 -> (64:128, 0), h2 -> (0:64, 1)
so the out-projection fuses h0+h1 into one K=128 matmul.
V_aug per-kb free layout [65 | 128 | 65]:
  h0: [v_h0, 1]; h1: [1, 0*63, v_h1] (y rows 64:128, denom row 0);
  h2: [v_h2, 1]
"""

import numpy as np

import concourse.bass as bass
import concourse.mybir as mybir
import concourse.tile as tile
from concourse import bacc
from concourse import bass_utils

P = 128
D = 64          # head dim
HPC = 3         # heads per core
C = 768
CK = C // P     # 6 contraction chunks
QK = 2 * HPC * D  # 384 (q+k cols per core)
NH = 12
B = 2
N_CORES = 8
F32 = mybir.dt.float32
MM_DT = mybir.dt.float32r

# (partition offset, chunk idx) per head, for q and k
Q_POS = [(0, 0), (0, 2), (64, 0)]
K_POS = [(0, 1), (0, 3), (64, 1)]
# wqk DRAM column ranges per chunk: (start, width)
QK_CHUNKS = [(0, 128), (128, 128), (256, 64), (320, 64)]
# V_aug free-layout per head: (lhsT start, lhsT width, denom row, y row0)
V_SLICE = [(0, 65, 64, 0), (65, 128, 0, 64), (193, 65, 64, 0)]
VW = 258
# yT destination (row0, chunk) per head
Y_POS = [(0, 0), (64, 0), (0, 1)]


def build_nc(T=2048, QCW=512):
    """Build the per-core Bass program. T = sequence length, QCW = q-chunk."""
    assert T % QCW == 0 and QCW % P == 0 and T % 512 == 0
    NQC = T // QCW
    NTB = T // P
    NPH = C // 2  # 384, out-proj free-dim half

    nc = bacc.Bacc("TRN2", target_bir_lowering=False, debug=False,
                   num_devices=N_CORES)
    xT = nc.dram_tensor("xT", [C, T], F32, kind="ExternalInput").ap()
    wqk = nc.dram_tensor("wqk", [C, QK], F32, kind="ExternalInput").ap()
    wv = nc.dram_tensor("wv", [C, HPC * D], F32, kind="ExternalInput").ap()
    bqk = nc.dram_tensor("bqk", [512], F32, kind="ExternalInput").ap()
    wp = nc.dram_tensor("wp", [2 * P, C], F32, kind="ExternalInput").ap()
    out = nc.dram_tensor("out", [T, C], F32, kind="ExternalOutput").ap()

    Exp = mybir.ActivationFunctionType.Exp

    with tile.TileContext(nc) as tc:
        with (
            tc.tile_pool(name="const", bufs=1) as const,
            tc.tile_pool(name="work", bufs=6) as work,
            tc.tile_pool(name="small", bufs=3) as small,
            tc.tile_pool(name="outp", bufs=4) as outp,
            tc.tile_pool(name="ps_mm", bufs=6, space="PSUM") as ps_mm,
            tc.tile_pool(name="ps_y", bufs=2, space="PSUM") as ps_y_pool,
        ):
            xT_sb = const.tile([P, CK, T], MM_DT, tag="xT")
            wqk_sb = const.tile([P, CK, QK], MM_DT, tag="wqk")
            wv_sb = const.tile([P, CK, HPC * D], MM_DT, tag="wv")
            bqk_sb = const.tile([P, 4], F32, tag="bqk")
            wp_sb = const.tile([P, 2, C], MM_DT, tag="wp")
            qkT_sb = const.tile([P, 4, T], MM_DT, tag="qkT")
            v_sb = const.tile([P, NTB, VW], MM_DT, tag="v")
            yT_sb = const.tile([P, 2, T], MM_DT, tag="yT")
            zb_sb = const.tile([P, 1], F32, tag="zb")
            ones_sb = const.tile([1, P], MM_DT, tag="ones")
            mask_sb = const.tile([P, 4, QCW], F32, tag="mask")
            zq_sb = const.tile([P, 512], F32, tag="zq")

            # ---- loads (weights first; xT per (kc, tj) chunk, spread
            # across the sync/gpsimd/scalar DMA queues) ----
            dma_engs = [nc.sync, nc.gpsimd, nc.scalar]
            # per-kc (wqk, xT-slice0) pairs interleaved at the head of all
            # three queues so the first qkT matmul chain starts ~2us in,
            # instead of waiting behind a monolithic 1.15MB wqk transfer
            nc.scalar.dma_start(bqk_sb[:], bqk.rearrange("(ci p) -> p ci", p=P))
            for kc in range(CK):
                dma_engs[(2 * kc) % 3].dma_start(
                    wqk_sb[:, kc, :],
                    wqk[kc * P:(kc + 1) * P, :].bitcast(MM_DT))
                dma_engs[(2 * kc + 1) % 3].dma_start(
                    xT_sb[:, kc, 0:512],
                    xT[kc * P:(kc + 1) * P, 0:512].bitcast(MM_DT))
            nc.gpsimd.dma_start(wv_sb[:], wv.bitcast(MM_DT).rearrange("(kc p) m -> p kc m", p=P))
            nc.scalar.dma_start(wp_sb[:], wp.bitcast(MM_DT).rearrange("(ci p) e -> p ci e", p=P))
            di = 0
            for tj in range(1, T // 512):
                for kc in range(CK):
                    dma_engs[di % 3].dma_start(
                        xT_sb[:, kc, tj * 512:(tj + 1) * 512],
                        xT[kc * P:(kc + 1) * P,
                           tj * 512:(tj + 1) * 512].bitcast(MM_DT))
                    di += 1

            # ---- constants (f32r tiles can't be memset; stage + cast) ----
            st = const.tile([P, 2], F32, tag="st")
            st1 = const.tile([1, P], F32, tag="st1")
            nc.gpsimd.memset(st[:], 1.0)
            nc.gpsimd.memset(st1[:], 1.0)
            nc.vector.tensor_copy(ones_sb[:], st1[:])
            nc.vector.tensor_copy(v_sb[:, :, 64:66],
                                  st[:, None, :].to_broadcast((P, NTB, 2)))
            nc.vector.tensor_copy(v_sb[:, :, 257:258],
                                  st[:, None, 0:1].to_broadcast((P, NTB, 1)))
            nc.gpsimd.memset(zq_sb[:], 0.0)
            # h1 junk cols (feed only never-read psy rows); zero for sim
            nc.vector.tensor_copy(v_sb[:, :, 66:129],
                                  zq_sb[:, None, 0:63].to_broadcast((P, NTB, 63)))
            nc.gpsimd.memset(zb_sb[:], 0.0)
            # causal 0/1 masks: mask_j[x, y] = 1 if y - x >= 128*j else 0
            nc.gpsimd.memset(mask_sb[:], 1.0)
            for j in range(4):
                nc.gpsimd.affine_select(
                    mask_sb[:, j, :], mask_sb[:, j, :],
                    pattern=[[1, QCW]],
                    compare_op=mybir.AluOpType.is_ge,
                    fill=0.0,
                    base=-128 * j,
                    channel_multiplier=-1,
                )

            # ---- qkT + V projections for one 512-token slice.
            # chunks 2+3 ([q1|k1]) fused into one 128-row matmul chain; the
            # k1 half is relocated to chunk 3 partitions 0:64 by an
            # SBUF->SBUF DMA (engines can't cross partitions, DMA can). ----
            def project_tj(tj):
                tjs = slice(tj * 512, (tj + 1) * 512)
                for ci in range(2):
                    ps = ps_mm.tile([P, 512], F32, tag="mm")
                    for kc in range(CK):
                        nc.tensor.matmul(
                            ps[:],
                            wqk_sb[:, kc, ci * P:(ci + 1) * P],
                            xT_sb[:, kc, tjs],
                            start=(kc == 0), stop=(kc == CK - 1),
                        )
                    nc.vector.tensor_scalar_add(
                        qkT_sb[:, ci, tjs], ps[:], bqk_sb[:, ci:ci + 1])
                ps = ps_mm.tile([P, 512], F32, tag="mm")
                for kc in range(CK):
                    nc.tensor.matmul(
                        ps[:],
                        wqk_sb[:, kc, 256:384],
                        xT_sb[:, kc, tjs],
                        start=(kc == 0), stop=(kc == CK - 1),
                    )
                nc.vector.tensor_scalar_add(
                    qkT_sb[0:D, 2, tjs], ps[0:D, :], bqk_sb[0:D, 2:3])
                qkst = work.tile([P, 512], MM_DT, tag="qkst")
                nc.vector.tensor_copy(qkst[D:P, :], ps[D:P, :])
                nc.sync.dma_start(qkT_sb[0:D, 3, tjs], qkst[D:P, :])
                for tb in range(4 * tj, 4 * tj + 4):
                    ps = ps_mm.tile([P, 512], F32, tag="mm")
                    for kc in range(CK):
                        nc.tensor.matmul(
                            ps[:, :HPC * D],
                            xT_sb[:, kc, tb * P:(tb + 1) * P],
                            wv_sb[:, kc, :],
                            start=(kc == 0), stop=(kc == CK - 1),
                        )
                    nc.vector.tensor_copy(v_sb[:, tb, 0:64], ps[:, 0:64])
                    nc.vector.tensor_copy(v_sb[:, tb, 129:257],
                                          ps[:, 64:192])

            # ---- attention (per q-chunk) and delayed out-projection.
            # Normalization of head i is emitted during head i+1's matmul
            # loop so its ACT->DVE->PE chain never stalls the in-order PE.
            pend1, pend2 = [], []

            def norm_stage1(st8):
                qc, h, psy_t = st8
                v0, vw, srow, yrow = V_SLICE[h]
                den = small.tile([1, QCW], F32, tag="den")
                nc.scalar.copy(den[:], psy_t[srow:srow + 1, :])
                recf = small.tile([1, QCW], F32, tag="recf")
                nc.vector.reciprocal_approx_fast(recf[:], den[:])
                recip = small.tile([1, QCW], MM_DT, tag="recip")
                nc.vector.tensor_copy(recip[:], recf[:])
                return (qc, h, psy_t, recip)

            def norm_stage2(st8):
                qc, h, psy_t, recip = st8
                q0 = qc * QCW
                v0, vw, srow, yrow = V_SLICE[h]
                psb = ps_mm.tile([P, QCW], F32, tag="mm", name="psb")
                nc.tensor.matmul(psb[:], ones_sb[:], recip[:],
                                 start=True, stop=True)
                bc = small.tile([P, QCW], F32, tag="bcs")
                yp, yci = Y_POS[h]
                nc.scalar.copy(bc[yrow:yrow + D, :],
                               psb[yrow:yrow + D, :])
                nc.vector.tensor_mul(
                    yT_sb[yp:yp + D, yci, q0:q0 + QCW],
                    psy_t[yrow:yrow + D, :], bc[yrow:yrow + D, :])

            def attn_qc(qc):
                q0 = qc * QCW
                kbmax = (q0 + QCW - 1) // P
                for h in range(HPC):
                    qp, qci = Q_POS[h]
                    kp, kci = K_POS[h]
                    v0, vw, srow, yrow = V_SLICE[h]
                    psy_t = ps_y_pool.tile([P, QCW], F32, tag="yaug",
                                           name="psy")
                    psy = psy_t[0:vw, :]
                    for kb in range(kbmax + 1):
                        # diagonal blocks only need q columns >= kb*128
                        n0 = max(0, kb * P - q0)
                        pss = ps_mm.tile([P, QCW], F32, tag="mm")
                        nc.tensor.matmul(
                            pss[:, n0:],
                            qkT_sb[kp:kp + D, kci, kb * P:(kb + 1) * P],
                            qkT_sb[qp:qp + D, qci, q0 + n0:q0 + QCW],
                            start=True, stop=True,
                        )
                        expT = work.tile([P, QCW], MM_DT, tag="expT")
                        nc.scalar.activation(expT[:, n0:], pss[:, n0:], Exp,
                                             bias=zb_sb[:])
                        if kb * P >= q0:  # diagonal block
                            nc.vector.tensor_mul(
                                expT[:, n0:], expT[:, n0:],
                                mask_sb[:, kb - q0 // P, n0:])
                        nc.tensor.matmul(
                            psy[:, n0:], v_sb[:, kb, v0:v0 + vw],
                            expT[:, n0:],
                            start=(kb == 0), stop=(kb == kbmax),
                        )
                        if kb == 0 and pend1:
                            pend2.append(norm_stage1(pend1.pop(0)))
                        if kb == 3 and pend2:
                            norm_stage2(pend2.pop(0))
                    pend1.append((qc, h, psy_t))

            def proj_qc(qc):
                q0 = qc * QCW
                for tb in range(q0 // P, (q0 + QCW) // P):
                    osb = outp.tile([P, C], F32, tag="osb")
                    for half in range(2):
                        pso = ps_mm.tile([P, 512], F32, tag="mm",
                                         name="pso")[:, :NPH]
                        nc.tensor.matmul(
                            pso, yT_sb[:, 0, tb * P:(tb + 1) * P],
                            wp_sb[:, 0, half * NPH:(half + 1) * NPH],
                            start=True, stop=False)
                        nc.tensor.matmul(
                            pso, yT_sb[0:D, 1, tb * P:(tb + 1) * P],
                            wp_sb[0:D, 1, half * NPH:(half + 1) * NPH],
                            start=False, stop=True)
                        nc.vector.tensor_copy(
                            osb[:, half * NPH:(half + 1) * NPH], pso)
                    dma_engs[tb % 3].dma_start(
                        out[tb * P:(tb + 1) * P, :], osb[:])

            # interleaved pipeline: projections(tj) -> attention(tj) ->
            # out-projection(tj-1), so PE never queues behind a later
            # slice's xT DMA
            for tj in range(T // 512):
                project_tj(tj)
                attn_qc(tj)
                if tj > 0:
                    proj_qc(tj - 1)
            while pend1:
                pend2.append(norm_stage1(pend1.pop(0)))
            while pend2:
                norm_stage2(pend2.pop(0))
            proj_qc(NQC - 1)


    nc.compile()
    return nc


_NC_CACHE = {}


def _get_nc(T=2048, QCW=512):
    key = (T, QCW)
    if key not in _NC_CACHE:
        _NC_CACHE[key] = build_nc(T, QCW)
    return _NC_CACHE[key]


def build_in_maps(inputs):
    """Build the 8 per-core input dicts from full inputs."""
    x = np.asarray(inputs["x"], np.float32)
    W = np.asarray(inputs["W_attn"], np.float32)
    b = np.asarray(inputs["b_attn"], np.float32)
    W_proj = np.asarray(inputs["W_proj"], np.float32)
    in_maps = []
    for c in range(N_CORES):
        bi, g = divmod(c, 4)
        lo = g * (HPC * D)  # local head col offset within each of q/k/v
        qw = [W[:, lo + i * D:lo + (i + 1) * D] * 0.125 for i in range(HPC)]
        kw = [W[:, C + lo + i * D:C + lo + (i + 1) * D] for i in range(HPC)]
        qb = [b[lo + i * D:lo + (i + 1) * D] * 0.125 for i in range(HPC)]
        # chunk order: [q0|q2], [k0|k2], [q1], [k1]
        wqk = np.concatenate([qw[0], qw[2], kw[0], kw[2], qw[1], kw[1]],
                             axis=1)
        z64 = np.zeros(D, np.float32)
        bqk = np.concatenate([qb[0], qb[2], z64, z64, qb[1], z64, z64, z64])
        wv = W[:, 2 * C + lo:2 * C + lo + HPC * D]
        # wp rows: [h0 | h1 | h2 | zero pad] -> chunks (0:128), (128:256)
        wp = np.zeros((2 * P, C), np.float32)
        wp[:HPC * D] = W_proj[lo:lo + HPC * D]
        in_maps.append({
            "xT": np.ascontiguousarray(x[bi].T),
            "wqk": np.ascontiguousarray(wqk),
            "wv": np.ascontiguousarray(wv),
            "bqk": np.ascontiguousarray(bqk),
            "wp": np.ascontiguousarray(wp),
        })
    return in_maps


def postprocess(results, inputs):
    b_attn = np.asarray(inputs["b_attn"], np.float32)
    W_proj = np.asarray(inputs["W_proj"], np.float32)
    b_proj = np.asarray(inputs["b_proj"], np.float32)
    b_eff = b_proj + b_attn[2 * C:] @ W_proj
    T = results[0]["out"].shape[0]
    out = np.zeros((B, T, C), np.float32)
    for c in range(N_CORES):
        out[c // 4] += results[c]["out"]
    out += b_eff
    return out


def kernel(x, W_attn, b_attn, W_proj, b_proj):
    inputs = dict(x=x, W_attn=W_attn, b_attn=b_attn,
                  W_proj=W_proj, b_proj=b_proj)
    T = np.asarray(x).shape[1]
    nc = _get_nc(T=T)
    in_maps = build_in_maps(inputs)
    res = bass_utils.run_bass_kernel_spmd(
        nc, in_maps, core_ids=list(range(N_CORES)))
    return postprocess(res.results, inputs)


# revision 34
# speedup vs baseline: 1.0623x; 1.0022x over previous
"""Causal self-attention (B=2, T=2048, C=768, H=12) on 8 Trainium2 cores.

Sharding: 24 (batch, head) pairs / 8 cores = 3 heads per core.
core c -> batch b = c // 4, heads [3g, 3g+3) with g = c % 4.

Per-core device program (identical SPMD program, different input data):
  qkT  = (Wqk_local^T @ x_b^T)          [384, T]   (q cols pre-scaled 1/8,
                                                    q bias added, k bias
                                                    dropped: softmax-invariant)
  V    = x_b @ Wv_local                  [T, 192]   (v bias folded on host)
  per head h:
    scoresT[k, q] = kT_h^T-block @ qT_h  (PE, K=64; diagonal blocks trimmed
                                          to the causally-needed q columns)
    expT = exp(scoresT)                  (ACT; diagonal blocks multiplied
                                          by precomputed 0/1 masks on DVE,
                                          trimmed cols zero-filled)
    y_augT[[d;1], q] += V_aug^T @ expT   (PE, ones row -> softmax denom)
    yT_h = y_augT[y rows] * (1/denom)    (DVE approx-reciprocal; denom
                                          broadcast via gpsimd
                                          partition_broadcast)
  out_partial = Y_local @ Wp_local       [T, 768]   (emitted one q-chunk
                                          late to avoid PE head-of-line
                                          blocking on the yT writes)

Host: out[b] = sum of the 4 partials + (b_proj + b_v @ W_proj).

Matmuls run in float32r (single-pass fp32, ~13 mantissa bits, ~2.2x
faster than the two-pass LOW_HIGH fp32 mode). Set MM_DT to
mybir.dt.float32 to go back to exact fp32.

qkT feature-chunk layout (matmul needs lhsT/rhs on the same base
partition, so each head's q and k live at the same partition offset):
  chunk0 = [q0 | q2], chunk1 = [k0 | k2], chunk2 = [q1], chunk3 = [k1]
yT layout [128, 2, T]: h0 -> (0:64, 0), h1 -> (64:128, 0), h2 -> (0:64, 1)
so the out-projection fuses h0+h1 into one K=128 matmul.
V_aug per-kb free layout [65 | 128 | 65]:
  h0: [v_h0, 1]; h1: [1, 0*63, v_h1] (y rows 64:128, denom row 0);
  h2: [v_h2, 1]
"""

import numpy as np

import concourse.bass as bass
import concourse.mybir as mybir
import concourse.tile as tile
from concourse import bacc
from concourse import bass_utils

P = 128
D = 64          # head dim
HPC = 3         # heads per core
C = 768
CK = C // P     # 6 contraction chunks
QK = 2 * HPC * D  # 384 (q+k cols per core)
NH = 12
B = 2
N_CORES = 8
F32 = mybir.dt.float32
MM_DT = mybir.dt.float32r

# (partition offset, chunk idx) per head, for q and k
Q_POS = [(0, 0), (0, 2), (64, 0)]
K_POS = [(0, 1), (0, 3), (64, 1)]
# wqk DRAM column ranges per chunk: (start, width)
QK_CHUNKS = [(0, 128), (128, 128), (256, 64), (320, 64)]
# V_aug free-layout per head: (lhsT start, lhsT width, denom row, y row0)
V_SLICE = [(0, 65, 64, 0), (65, 128, 0, 64), (193, 65, 64, 0)]
VW = 258
# yT destination (row0, chunk) per head
Y_POS = [(0, 0), (64, 0), (0, 1)]


def build_nc(T=2048, QCW=512):
    """Build the per-core Bass program. T = sequence length, QCW = q-chunk."""
    assert T % QCW == 0 and QCW % P == 0 and T % 512 == 0
    NQC = T // QCW
    NTB = T // P
    NPH = C // 2  # 384, out-proj free-dim half

    nc = bacc.Bacc("TRN2", target_bir_lowering=False, debug=False,
                   num_devices=N_CORES)
    xT = nc.dram_tensor("xT", [C, T], F32, kind="ExternalInput").ap()
    wqk = nc.dram_tensor("wqk", [C, QK], F32, kind="ExternalInput").ap()
    wv = nc.dram_tensor("wv", [C, HPC * D], F32, kind="ExternalInput").ap()
    bqk = nc.dram_tensor("bqk", [512], F32, kind="ExternalInput").ap()
    wp = nc.dram_tensor("wp", [2 * P, C], F32, kind="ExternalInput").ap()
    out = nc.dram_tensor("out", [T, C], F32, kind="ExternalOutput").ap()

    Exp = mybir.ActivationFunctionType.Exp

    with tile.TileContext(nc) as tc:
        with (
            tc.tile_pool(name="const", bufs=1) as const,
            tc.tile_pool(name="work", bufs=8) as work,
            tc.tile_pool(name="small", bufs=3) as small,
            tc.tile_pool(name="outp", bufs=3) as outp,
            tc.tile_pool(name="ps_mm", bufs=6, space="PSUM") as ps_mm,
            tc.tile_pool(name="ps_y", bufs=2, space="PSUM") as ps_y_pool,
        ):
            xT_sb = const.tile([P, CK, T], MM_DT, tag="xT")
            wqk_sb = const.tile([P, CK, QK], MM_DT, tag="wqk")
            wv_sb = const.tile([P, CK, HPC * D], MM_DT, tag="wv")
            bqk_sb = const.tile([P, 4], F32, tag="bqk")
            wp_sb = const.tile([P, 2, C], MM_DT, tag="wp")
            qkT_sb = const.tile([P, 4, T], MM_DT, tag="qkT")
            v_sb = const.tile([P, NTB, VW], MM_DT, tag="v")
            yT_sb = const.tile([P, 2, T], MM_DT, tag="yT")
            zb_sb = const.tile([P, 1], F32, tag="zb")
            ones_sb = const.tile([1, P], MM_DT, tag="ones")
            mask_sb = const.tile([P, 4, QCW], F32, tag="mask")
            zq_sb = const.tile([P, 64], F32, tag="zq")

            # ---- loads (weights first; xT per (kc, tj) chunk, spread
            # across the sync/gpsimd/scalar DMA queues) ----
            dma_engs = [nc.sync, nc.gpsimd, nc.scalar]
            # per-kc (wqk, xT-slice0) pairs interleaved at the head of all
            # three queues so the first qkT matmul chain starts ~2us in,
            # instead of waiting behind a monolithic 1.15MB wqk transfer
            nc.scalar.dma_start(bqk_sb[:], bqk.rearrange("(ci p) -> p ci", p=P))
            for kc in range(CK):
                dma_engs[(2 * kc) % 3].dma_start(
                    wqk_sb[:, kc, :],
                    wqk[kc * P:(kc + 1) * P, :].bitcast(MM_DT))
                dma_engs[(2 * kc + 1) % 3].dma_start(
                    xT_sb[:, kc, 0:512],
                    xT[kc * P:(kc + 1) * P, 0:512].bitcast(MM_DT))
            nc.gpsimd.dma_start(wv_sb[:], wv.bitcast(MM_DT).rearrange("(kc p) m -> p kc m", p=P))
            nc.scalar.dma_start(wp_sb[:], wp.bitcast(MM_DT).rearrange("(ci p) e -> p ci e", p=P))
            di = 0
            for tj in range(1, T // 512):
                for kc in range(CK):
                    dma_engs[di % 3].dma_start(
                        xT_sb[:, kc, tj * 512:(tj + 1) * 512],
                        xT[kc * P:(kc + 1) * P,
                           tj * 512:(tj + 1) * 512].bitcast(MM_DT))
                    di += 1

            # ---- constants (f32r tiles can't be memset; stage + cast) ----
            st = const.tile([P, 2], F32, tag="st")
            st1 = const.tile([1, P], F32, tag="st1")
            nc.gpsimd.memset(st[:], 1.0)
            nc.gpsimd.memset(st1[:], 1.0)
            nc.vector.tensor_copy(ones_sb[:], st1[:])
            nc.vector.tensor_copy(v_sb[:, :, 64:66],
                                  st[:, None, :].to_broadcast((P, NTB, 2)))
            nc.vector.tensor_copy(v_sb[:, :, 257:258],
                                  st[:, None, 0:1].to_broadcast((P, NTB, 1)))
            nc.gpsimd.memset(zq_sb[:], 0.0)
            # h1 junk cols (feed only never-read psy rows); zero for sim
            nc.vector.tensor_copy(v_sb[:, :, 66:129],
                                  zq_sb[:, None, 0:63].to_broadcast((P, NTB, 63)))
            nc.gpsimd.memset(zb_sb[:], 0.0)
            # causal 0/1 masks: mask_j[x, y] = 1 if y - x >= 128*j else 0
            nc.gpsimd.memset(mask_sb[:], 1.0)
            for j in range(4):
                nc.gpsimd.affine_select(
                    mask_sb[:, j, :], mask_sb[:, j, :],
                    pattern=[[1, QCW]],
                    compare_op=mybir.AluOpType.is_ge,
                    fill=0.0,
                    base=-128 * j,
                    channel_multiplier=-1,
                )

            # ---- qkT + V projections for one 512-token slice.
            # chunks 2+3 ([q1|k1]) fused into one 128-row matmul chain; the
            # k1 half is relocated to chunk 3 partitions 0:64 by an
            # SBUF->SBUF DMA (engines can't cross partitions, DMA can). ----
            def project_tj(tj):
                tjs = slice(tj * 512, (tj + 1) * 512)
                for ci in range(2):
                    ps = ps_mm.tile([P, 512], F32, tag="mm")
                    for kc in range(CK):
                        nc.tensor.matmul(
                            ps[:],
                            wqk_sb[:, kc, ci * P:(ci + 1) * P],
                            xT_sb[:, kc, tjs],
                            start=(kc == 0), stop=(kc == CK - 1),
                        )
                    nc.vector.tensor_scalar_add(
                        qkT_sb[:, ci, tjs], ps[:], bqk_sb[:, ci:ci + 1])
                ps = ps_mm.tile([P, 512], F32, tag="mm")
                for kc in range(CK):
                    nc.tensor.matmul(
                        ps[:],
                        wqk_sb[:, kc, 256:384],
                        xT_sb[:, kc, tjs],
                        start=(kc == 0), stop=(kc == CK - 1),
                    )
                nc.vector.tensor_scalar_add(
                    qkT_sb[0:D, 2, tjs], ps[0:D, :], bqk_sb[0:D, 2:3])
                qkst = work.tile([P, 512], MM_DT, tag="qkst")
                nc.vector.tensor_copy(qkst[D:P, :], ps[D:P, :])
                nc.sync.dma_start(qkT_sb[0:D, 3, tjs], qkst[D:P, :])
                for tb in range(4 * tj, 4 * tj + 4):
                    ps = ps_mm.tile([P, 512], F32, tag="mm")
                    for kc in range(CK):
                        nc.tensor.matmul(
                            ps[:, :HPC * D],
                            xT_sb[:, kc, tb * P:(tb + 1) * P],
                            wv_sb[:, kc, :],
                            start=(kc == 0), stop=(kc == CK - 1),
                        )
                    nc.vector.tensor_copy(v_sb[:, tb, 0:64], ps[:, 0:64])
                    nc.vector.tensor_copy(v_sb[:, tb, 129:257],
                                          ps[:, 64:192])

            # ---- attention (per q-chunk) and delayed out-projection.
            # Normalization of head i is emitted during head i+1's matmul
            # loop so its ACT->DVE->PE chain never stalls the in-order PE.
            pend1, pend2 = [], []

            def norm_stage1(st8):
                qc, h, psy_t = st8
                v0, vw, srow, yrow = V_SLICE[h]
                den = small.tile([1, QCW], F32, tag="den")
                nc.scalar.copy(den[:], psy_t[srow:srow + 1, :])
                recf = small.tile([1, QCW], F32, tag="recf")
                nc.vector.reciprocal_approx_fast(recf[:], den[:])
                recip = small.tile([1, QCW], MM_DT, tag="recip")
                nc.vector.tensor_copy(recip[:], recf[:])
                return (qc, h, psy_t, recip)

            def norm_stage2(st8):
                qc, h, psy_t, recip = st8
                q0 = qc * QCW
                v0, vw, srow, yrow = V_SLICE[h]
                psb = ps_mm.tile([P, QCW], F32, tag="mm", name="psb")
                nc.tensor.matmul(psb[:], ones_sb[:], recip[:],
                                 start=True, stop=True)
                bc = small.tile([P, QCW], F32, tag="bcs")
                yp, yci = Y_POS[h]
                nc.scalar.copy(bc[yrow:yrow + D, :],
                               psb[yrow:yrow + D, :])
                nc.vector.tensor_mul(
                    yT_sb[yp:yp + D, yci, q0:q0 + QCW],
                    psy_t[yrow:yrow + D, :], bc[yrow:yrow + D, :])

            def attn_qc(qc):
                q0 = qc * QCW
                kbmax = (q0 + QCW - 1) // P
                for h in range(HPC):
                    qp, qci = Q_POS[h]
                    kp, kci = K_POS[h]
                    v0, vw, srow, yrow = V_SLICE[h]
                    psy_t = ps_y_pool.tile([P, QCW], F32, tag="yaug",
                                           name="psy")
                    psy = psy_t[0:vw, :]
                    for kb in range(kbmax + 1):
                        # diagonal blocks only need q columns >= kb*128
                        n0 = max(0, kb * P - q0)
                        pss = ps_mm.tile([P, QCW], F32, tag="mm")
                        nc.tensor.matmul(
                            pss[:, n0:],
                            qkT_sb[kp:kp + D, kci, kb * P:(kb + 1) * P],
                            qkT_sb[qp:qp + D, qci, q0 + n0:q0 + QCW],
                            start=True, stop=True,
                        )
                        expT = work.tile([P, QCW], MM_DT, tag="expT")
                        nc.scalar.activation(expT[:, n0:], pss[:, n0:], Exp,
                                             bias=zb_sb[:])
                        if kb * P >= q0:  # diagonal block
                            nc.vector.tensor_mul(
                                expT[:, n0:], expT[:, n0:],
                                mask_sb[:, kb - q0 // P, n0:])
                        nc.tensor.matmul(
                            psy[:, n0:], v_sb[:, kb, v0:v0 + vw],
                            expT[:, n0:],
                            start=(kb == 0), stop=(kb == kbmax),
                        )
                        if kb == 0 and pend1:
                            pend2.append(norm_stage1(pend1.pop(0)))
                        if kb == 3 and pend2:
                            norm_stage2(pend2.pop(0))
                    pend1.append((qc, h, psy_t))

            def proj_qc(qc):
                q0 = qc * QCW
                for tb in range(q0 // P, (q0 + QCW) // P):
                    osb = outp.tile([P, C], F32, tag="osb")
                    for half in range(2):
                        pso = ps_mm.tile([P, 512], F32, tag="mm",
                                         name="pso")[:, :NPH]
                        nc.tensor.matmul(
                            pso, yT_sb[:, 0, tb * P:(tb + 1) * P],
                            wp_sb[:, 0, half * NPH:(half + 1) * NPH],
                            start=True, stop=False)
                        nc.tensor.matmul(
                            pso, yT_sb[0:D, 1, tb * P:(tb + 1) * P],
                            wp_sb[0:D, 1, half * NPH:(half + 1) * NPH],
                            start=False, stop=True)
                        nc.vector.tensor_copy(
                            osb[:, half * NPH:(half + 1) * NPH], pso)
                    dma_engs[tb % 3].dma_start(
                        out[tb * P:(tb + 1) * P, :], osb[:])

            # interleaved pipeline: projections(tj) -> attention(tj) ->
            # out-projection(tj-1), so PE never queues behind a later
            # slice's xT DMA
            for tj in range(T // 512):
                project_tj(tj)
                attn_qc(tj)
                if tj > 0:
                    proj_qc(tj - 1)
            while pend1:
                pend2.append(norm_stage1(pend1.pop(0)))
            while pend2:
                norm_stage2(pend2.pop(0))
            proj_qc(NQC - 1)


    nc.compile()
    return nc


_NC_CACHE = {}


def _get_nc(T=2048, QCW=512):
    key = (T, QCW)
    if key not in _NC_CACHE:
        _NC_CACHE[key] = build_nc(T, QCW)
    return _NC_CACHE[key]


def build_in_maps(inputs):
    """Build the 8 per-core input dicts from full inputs."""
    x = np.asarray(inputs["x"], np.float32)
    W = np.asarray(inputs["W_attn"], np.float32)
    b = np.asarray(inputs["b_attn"], np.float32)
    W_proj = np.asarray(inputs["W_proj"], np.float32)
    in_maps = []
    for c in range(N_CORES):
        bi, g = divmod(c, 4)
        lo = g * (HPC * D)  # local head col offset within each of q/k/v
        qw = [W[:, lo + i * D:lo + (i + 1) * D] * 0.125 for i in range(HPC)]
        kw = [W[:, C + lo + i * D:C + lo + (i + 1) * D] for i in range(HPC)]
        qb = [b[lo + i * D:lo + (i + 1) * D] * 0.125 for i in range(HPC)]
        # chunk order: [q0|q2], [k0|k2], [q1], [k1]
        wqk = np.concatenate([qw[0], qw[2], kw[0], kw[2], qw[1], kw[1]],
                             axis=1)
        z64 = np.zeros(D, np.float32)
        bqk = np.concatenate([qb[0], qb[2], z64, z64, qb[1], z64, z64, z64])
        wv = W[:, 2 * C + lo:2 * C + lo + HPC * D]
        # wp rows: [h0 | h1 | h2 | zero pad] -> chunks (0:128), (128:256)
        wp = np.zeros((2 * P, C), np.float32)
        wp[:HPC * D] = W_proj[lo:lo + HPC * D]
        in_maps.append({
            "xT": np.ascontiguousarray(x[bi].T),
            "wqk": np.ascontiguousarray(wqk),
            "wv": np.ascontiguousarray(wv),
            "bqk": np.ascontiguousarray(bqk),
            "wp": np.ascontiguousarray(wp),
        })
    return in_maps


def postprocess(results, inputs):
    b_attn = np.asarray(inputs["b_attn"], np.float32)
    W_proj = np.asarray(inputs["W_proj"], np.float32)
    b_proj = np.asarray(inputs["b_proj"], np.float32)
    b_eff = b_proj + b_attn[2 * C:] @ W_proj
    T = results[0]["out"].shape[0]
    out = np.zeros((B, T, C), np.float32)
    for c in range(N_CORES):
        out[c // 4] += results[c]["out"]
    out += b_eff
    return out


def kernel(x, W_attn, b_attn, W_proj, b_proj):
    inputs = dict(x=x, W_attn=W_attn, b_attn=b_attn,
                  W_proj=W_proj, b_proj=b_proj)
    T = np.asarray(x).shape[1]
    nc = _get_nc(T=T)
    in_maps = build_in_maps(inputs)
    res = bass_utils.run_bass_kernel_spmd(
        nc, in_maps, core_ids=list(range(N_CORES)))
    return postprocess(res.results, inputs)
